# revision 16
# baseline (speedup 1.0000x reference)
"""Multi-head attention (B=4, S=2048, D=1024, H=16) on 8 Trainium2 cores.

Sharding: (batch, head-group) grid — core c handles batch c//2, heads
(c%2)*8..(c%2)*8+8. Zero duplicated FLOPs; host sums the two partial
out-projections per batch and adds bo.

Per-core kernel (fp32 data, matmuls in float32r = FP22 1-pass, full rate):
  phase A: K^T/Q^T [512,2048] (feature-major) + V [2048,8x65] (token-major,
           ones-augmented per head) projections from host-pre-transposed X^T.
           Emission order: K pair0, Q pair0 qb0, V, rest — so the attention
           exp stream starts ~12us in instead of after all projections.
  phase B: per (head-pair, q-block of 1024):
           S^T[k,q] = K.Q^T via row-packed C=64 matmuls (2 heads concurrent
           on PE row-groups 0-63/64-127),
           P^T = exp(S^T/8) on ScalarE straight out of PSUM ([128,1024]
           activations = near peak rate),
           AO^T_aug[65,q] += V_aug^T.P^T accumulated over 16 k-tiles
           (row 64 = softmax denominator).
           Copy AO to SBUF immediately (frees PSUM fast, keeps PE warm);
           reciprocal of the denominator reshaped to [128,8] (full-lane),
           broadcast across partitions via DRAM bounce, normalize on DVE.
  phase C: out-proj per q-block, token-major [2048,1024] partial to HBM;
           emission interleaved into the next q-block's pairs.
"""

import numpy as np

import bass_rust
import concourse.bass as bass
import concourse.tile as tile
from concourse import mybir

F32 = mybir.dt.float32
F32R = mybir.dt.float32r

B, S, D = 4, 2048, 1024
NH, DK = 16, 64          # total heads, head dim
HG = 8                   # heads per core (head group)
DHG = HG * DK            # 512 features per head group
NP = 4                   # pairs of heads per core
QB = 1024                # q-block size
NQB = S // QB            # 2
KT = S // 128            # 16 k-tiles
CT = D // 128            # 8 contraction chunks for projections
VW = DK + 1              # 65: V columns per head incl. ones column


def split_multi_waits(nc):
    """This toolchain's walrus accepts only ONE sync-wait per instruction;
    Tile attaches several (one per producer proc). Hoist all but one wait
    onto single-wait NOPs inserted just before the instruction on the same
    engine (engines are in-order, so semantics are identical)."""
    uid = 0
    for f in nc.m.functions:
        for bb in f.blocks:
            il = bb.instructions
            i = 0
            while i < len(il):
                inst = il[i]
                si = inst.sync_info
                if si is not None and len(si.on_wait) > 1:
                    waits = list(si.on_wait)
                    inst.sync_info = bass_rust.SyncInfo(
                        on_wait=[waits[-1]], on_update=list(si.on_update)
                    )
                    for w in waits[:-1]:
                        nop = mybir.InstNoOp(
                            name=f"WSPLIT-{uid}",
                            engine=inst.engine,
                            bass_nofuse=True,
                            sync_info=bass_rust.SyncInfo(
                                on_wait=[w], on_update=[]
                            ),
                        )
                        uid += 1
                        il.insert(i, nop)
                        i += 1
                i += 1


def bcast_ap(ap, parts, n):
    """Partition-broadcast view of a DRAM row AP: [[0,parts],[1,n]]."""
    return bass.AP(tensor=ap.tensor, offset=ap.offset, ap=[[0, parts], [1, n]])


def build_kernel():
    nc = bass.Bass(trn_type="TRN2")

    xq = nc.dram_tensor("xq", (D, S), F32R, kind="ExternalInput")   # query[b].T
    xk = nc.dram_tensor("xk", (D, S), F32R, kind="ExternalInput")
    xv = nc.dram_tensor("xv", (D, S), F32R, kind="ExternalInput")
    wq = nc.dram_tensor("wq", (D, DHG), F32R, kind="ExternalInput")  # Wq[hg].T
    wk = nc.dram_tensor("wk", (D, DHG), F32R, kind="ExternalInput")
    wv = nc.dram_tensor("wv", (D, DHG), F32R, kind="ExternalInput")
    wo = nc.dram_tensor("wo", (DHG, D), F32R, kind="ExternalInput")  # Wo[:,hg].T
    bq = nc.dram_tensor("bq", (DHG,), F32, kind="ExternalInput")
    bk = nc.dram_tensor("bk", (DHG,), F32, kind="ExternalInput")
    bv = nc.dram_tensor("bv", (DHG,), F32, kind="ExternalInput")
    out = nc.dram_tensor("out", (S, D), F32, kind="ExternalOutput")

    from contextlib import ExitStack

    with tile.TileContext(nc) as tc, ExitStack() as ctx:
        persist = ctx.enter_context(tc.tile_pool(name="persist", bufs=1))
        KT_sb = persist.tile([128, NP, S], F32R)       # K^T: pair p rows
        QT_sb = persist.tile([128, NP, S], F32R)       # Q^T
        V_sb = persist.tile([128, KT, HG, VW], F32R)   # V token-major + ones
        AON = persist.tile([128, NP, S], F32R)         # normalized AO^T
        bq_sb = persist.tile([128, NP], F32)
        bk_sb = persist.tile([128, NP], F32)
        bv_bc = persist.tile([128, DHG], F32)          # bv broadcast

        with nc.allow_non_contiguous_dma(reason="tiny bias loads"):
            nc.sync.dma_start(bq_sb[:], bq.rearrange("(t p) -> p t", p=128))
            nc.sync.dma_start(bk_sb[:], bk.rearrange("(t p) -> p t", p=128))
        nc.sync.dma_start(bv_bc[:], bcast_ap(bv[:], 128, DHG))
        nc.vector.memset(V_sb[:, :, :, DK].bitcast(F32), 1.0)   # ones columns

        # ---- phase A: projections ------------------------------------------
        phase_a = ExitStack()
        wpool = phase_a.enter_context(tc.tile_pool(name="wts", bufs=1))
        xpool = phase_a.enter_context(tc.tile_pool(name="xstream", bufs=3))
        pproj = phase_a.enter_context(
            tc.tile_pool(name="pproj", bufs=1, space="PSUM")
        )

        wk_sb = wpool.tile([128, CT, DHG], F32R)
        wq_sb = wpool.tile([128, CT, DHG], F32R)
        wv_sb = wpool.tile([128, CT, DHG], F32R)
        nc.sync.dma_start(wk_sb[:], wk.rearrange("(c p) n -> p c n", p=128))
        nc.sync.dma_start(wq_sb[:], wq.rearrange("(c p) n -> p c n", p=128))
        nc.sync.dma_start(wv_sb[:], wv.rearrange("(c p) n -> p c n", p=128))

        def kq_pass(xdram, w_sb, dst, b_sb, jts, khs):
            """dst[dout 128, q] += w[:,ct,jt].T @ x^T[ct, q] for jt in jts."""
            for kh in khs:
                ps = {
                    jt: pproj.tile([128, QB], F32, tag=f"proj{jt}",
                                   name=f"proj{jt}")
                    for jt in jts
                }
                for ct in range(CT):
                    xc = xpool.tile([128, QB], F32R, tag="xchunk", name="xc")
                    nc.sync.dma_start(
                        xc[:], xdram[ct * 128:(ct + 1) * 128,
                                     kh * QB:(kh + 1) * QB]
                    )
                    for jt in jts:
                        for qc in range(QB // 512):
                            nc.tensor.matmul(
                                ps[jt][:, qc * 512:(qc + 1) * 512],
                                w_sb[:, ct, jt * 128:(jt + 1) * 128],
                                xc[:, qc * 512:(qc + 1) * 512],
                                start=(ct == 0), stop=(ct == CT - 1),
                            )
                for jt in jts:
                    nc.vector.tensor_scalar_add(
                        dst[:, jt, kh * QB:(kh + 1) * QB],
                        ps[jt][:],
                        b_sb[:, jt:jt + 1],
                    )

        def v_pass():
            """V_sb[tok, h, 0:64] += x^T[ct, tok].T @ wv[:, ct, :], + bias."""
            for q4 in range(4):  # quarters of the token dim (4 tok-tiles)
                ps = {
                    i: pproj.tile([128, DHG], F32, tag=f"proj{i}",
                                  name=f"vproj{i}")
                    for i in range(4)
                }
                for ct in range(CT):
                    xc = xpool.tile([128, 512], F32R, tag="xchunk", name="xcv")
                    nc.sync.dma_start(
                        xc[:], xv[ct * 128:(ct + 1) * 128,
                                  q4 * 512:(q4 + 1) * 512]
                    )
                    for i in range(4):
                        nc.tensor.matmul(
                            ps[i][:],
                            xc[:, i * 128:(i + 1) * 128],
                            wv_sb[:, ct, :],
                            start=(ct == 0), stop=(ct == CT - 1),
                        )
                for i in range(4):
                    vtile = q4 * 4 + i
                    nc.vector.tensor_add(
                        V_sb[:, vtile, :, 0:DK],
                        ps[i][:].rearrange("p (h d) -> p h d", d=DK),
                        bv_bc[:].rearrange("p (h d) -> p h d", d=DK),
                    )

        # pair-0 K and q-block-0 Q first so attention starts early
        kq_pass(xk, wk_sb, KT_sb, bk_sb, jts=[0], khs=[0, 1])
        kq_pass(xq, wq_sb, QT_sb, bq_sb, jts=[0], khs=[0])
        v_pass()
        kq_pass(xk, wk_sb, KT_sb, bk_sb, jts=[1, 2, 3], khs=[0, 1])
        kq_pass(xq, wq_sb, QT_sb, bq_sb, jts=[1, 2, 3], khs=[0])
        kq_pass(xq, wq_sb, QT_sb, bq_sb, jts=[0, 1, 2, 3], khs=[1])

        phase_a.close()

        # ---- phases B+C: attention + out-projection ------------------------
        wopool = ctx.enter_context(tc.tile_pool(name="wopool", bufs=1))
        wo_sb = wopool.tile([128, NP, D], F32R)        # out-proj weights
        nc.sync.dma_start(wo_sb[:], wo.rearrange("(c p) n -> p c n", p=128))

        pmm = ctx.enter_context(tc.tile_pool(name="pmm", bufs=1, space="PSUM"))
        ptp = ctx.enter_context(tc.tile_pool(name="ptile", bufs=5))
        npool = ctx.enter_context(tc.tile_pool(name="norm", bufs=3))
        opool = ctx.enter_context(tc.tile_pool(name="ostage", bufs=2))
        dpool = ctx.enter_context(
            tc.tile_pool(name="dscratch", bufs=3, space="DRAM")
        )

        def outproj_tile(qb, tt):
            """Emit out-projection for token tile tt of q-block qb."""
            q0 = qb * QB
            ot = opool.tile([128, D], F32, tag="ot", name="ot")
            po = pmm.tile([128, QB], F32, tag=f"ao{tt % 2}", name="po")
            for oh in range(2):
                for ci in range(NP):
                    nc.tensor.matmul(
                        po[:, oh * 512:(oh + 1) * 512],
                        AON[:, ci, q0 + tt * 128:q0 + (tt + 1) * 128],
                        wo_sb[:, ci, oh * 512:(oh + 1) * 512],
                        start=(ci == 0), stop=(ci == NP - 1),
                    )
            nc.vector.tensor_copy(ot[:], po[:])
            nc.sync.dma_start(out[q0 + tt * 128:q0 + (tt + 1) * 128, :], ot[:])

        for qb in range(NQB):
            q0 = qb * QB
            for p in range(NP):
                ao = [
                    pmm.tile([VW, QB], F32, tag=f"ao{h2}", name=f"ao{h2}")
                    for h2 in range(2)
                ]
                for kt in range(KT):
                    for h2 in range(2):
                        hh = 2 * p + h2
                        lo, hi = h2 * DK, h2 * DK + DK
                        st = pmm.tile([128, QB], F32, tag="st", name="st",
                                      bufs=2)
                        for qc in range(QB // 512):
                            nc.tensor.matmul(
                                st[:, qc * 512:(qc + 1) * 512],
                                KT_sb[lo:hi, p, kt * 128:(kt + 1) * 128],
                                QT_sb[lo:hi, p,
                                      q0 + qc * 512:q0 + (qc + 1) * 512],
                                start=True, stop=True,
                            )
                        pt = ptp.tile([128, QB], F32R, tag="pt", name="pt")
                        nc.scalar.activation(
                            pt[:], st[:],
                            mybir.ActivationFunctionType.Exp,
                            scale=0.125,
                        )
                        for qc in range(QB // 512):
                            nc.tensor.matmul(
                                ao[h2][:, qc * 512:(qc + 1) * 512],
                                V_sb[:, kt, hh, :],
                                pt[:, qc * 512:(qc + 1) * 512],
                                start=(kt == 0), stop=(kt == KT - 1),
                            )
                for h2 in range(2):
                    # copy to SBUF promptly: frees the PSUM slot so the next
                    # pair's PV stream never stalls the PE
                    aos = npool.tile([VW, QB], F32, tag="aos", name="aos")
                    nc.vector.tensor_copy(aos[:], ao[h2][:])
                    # full-lane reciprocal of the denominator via a
                    # [1,1024] -> [128,8] DRAM-bounce reshape
                    dn = dpool.tile([1, QB], F32, tag="dn", name="dn")
                    nc.sync.dma_start(dn[:], aos[DK:VW, :])
                    rc = npool.tile([128, 8], F32, tag="rc", name="rc")
                    nc.sync.dma_start(
                        rc[:], dn[:].rearrange("x (p j) -> (x p) j", j=8)
                    )
                    nc.vector.reciprocal(rc[:], rc[:])
                    rcd = dpool.tile([1, QB], F32, tag="rcd", name="rcd")
                    nc.sync.dma_start(
                        rcd[:].rearrange("x (p j) -> (x p) j", j=8), rc[:]
                    )
                    rb = npool.tile([DK, QB], F32, tag="rb", name="rb")
                    nc.sync.dma_start(rb[:], bcast_ap(rcd[:], DK, QB))
                    nc.vector.tensor_mul(
                        AON[h2 * DK:(h2 + 1) * DK, p, q0:q0 + QB],
                        aos[0:DK, :],
                        rb[:],
                    )
                # interleave previous q-block's out-projection
                if qb > 0:
                    for tt in range(2 * p, 2 * p + 2):
                        outproj_tile(qb - 1, tt)
        for tt in range(QB // 128):
            outproj_tile(NQB - 1, tt)

    split_multi_waits(nc)
    return nc


def _prep_inputs(query, key, value, Wq, bq, Wk, bk, Wv, bv, Wo, bo):
    """Build the 8 per-core input maps."""
    xt = {}
    for nm, x in (("xq", query), ("xk", key), ("xv", value)):
        xt[nm] = [np.ascontiguousarray(x[b].T) for b in range(B)]
    in_maps = []
    for c in range(8):
        b, g = divmod(c, 2)
        rows = slice(g * DHG, (g + 1) * DHG)
        in_maps.append({
            "xq": xt["xq"][b], "xk": xt["xk"][b], "xv": xt["xv"][b],
            "wq": np.ascontiguousarray(Wq[rows, :].T),
            "wk": np.ascontiguousarray(Wk[rows, :].T),
            "wv": np.ascontiguousarray(Wv[rows, :].T),
            "wo": np.ascontiguousarray(Wo[:, rows].T),
            "bq": np.ascontiguousarray(bq[rows]),
            "bk": np.ascontiguousarray(bk[rows]),
            "bv": np.ascontiguousarray(bv[rows]),
        })
    return in_maps


_NC_CACHE = None


def run(inputs, trace=False):
    """Returns (full_output, BassKernelResults)."""
    global _NC_CACHE
    from concourse.bass_utils import run_bass_kernel_spmd

    inputs = {k: np.asarray(v, np.float32) for k, v in inputs.items()}
    in_maps = _prep_inputs(**inputs)
    if _NC_CACHE is None:
        _NC_CACHE = build_kernel()
    res = run_bass_kernel_spmd(
        _NC_CACHE, in_maps, core_ids=list(range(8)), trace=trace
    )
    bo = inputs["bo"]
    full = np.empty((B, S, D), np.float32)
    for b in range(B):
        full[b] = res.results[2 * b]["out"] + res.results[2 * b + 1]["out"] + bo
    return full, res


def kernel(**inputs):
    return run(inputs, trace=False)[0]


# revision 17
# speedup vs baseline: 1.1143x; 1.1143x over previous
"""Multi-head attention (B=4, S=2048, D=1024, H=16) on 8 Trainium2 cores.

Sharding: (batch, head-group) grid — core c handles batch c//2, heads
(c%2)*8..(c%2)*8+8. Zero duplicated FLOPs; host sums the two partial
out-projections per batch and adds bo.

Per-core kernel (fp32 data, matmuls in float32r = FP22 1-pass, full rate):
  phase A: K^T/Q^T [512,2048] (feature-major) + V [2048,8x65] (token-major,
           ones-augmented per head) projections from host-pre-transposed X^T.
           Emission order: K pair0, Q pair0 qb0, V, rest — so the attention
           exp stream starts ~12us in instead of after all projections.
  phase B: per (head-pair, q-block of 1024):
           S^T[k,q] = K.Q^T via row-packed C=64 matmuls (2 heads concurrent
           on PE row-groups 0-63/64-127),
           P^T = exp(S^T/8) on ScalarE straight out of PSUM ([128,1024]
           activations = near peak rate),
           AO^T_aug[65,q] += V_aug^T.P^T accumulated over 16 k-tiles
           (row 64 = softmax denominator).
           Copy AO to SBUF immediately (frees PSUM fast, keeps PE warm);
           reciprocal of the denominator reshaped to [128,8] (full-lane),
           broadcast across partitions via DRAM bounce, normalize on DVE.
  phase C: out-proj per q-block, token-major [2048,1024] partial to HBM;
           emission interleaved into the next q-block's pairs.
"""

import numpy as np

import bass_rust
import concourse.bass as bass
import concourse.tile as tile
from concourse import mybir

F32 = mybir.dt.float32
F32R = mybir.dt.float32r
BF16 = mybir.dt.bfloat16
USE_BF16 = True
MMD = BF16 if USE_BF16 else F32R

B, S, D = 4, 2048, 1024
NH, DK = 16, 64          # total heads, head dim
HG = 8                   # heads per core (head group)
DHG = HG * DK            # 512 features per head group
NP = 4                   # pairs of heads per core
QB = 1024                # q-block size
NQB = S // QB            # 2
KT = S // 128            # 16 k-tiles
CT = D // 128            # 8 contraction chunks for projections
VW = DK + 1              # 65: V columns per head incl. ones column


def split_multi_waits(nc):
    """This toolchain's walrus accepts only ONE sync-wait per instruction;
    Tile attaches several (one per producer proc). Hoist all but one wait
    onto single-wait NOPs inserted just before the instruction on the same
    engine (engines are in-order, so semantics are identical)."""
    uid = 0
    for f in nc.m.functions:
        for bb in f.blocks:
            il = bb.instructions
            i = 0
            while i < len(il):
                inst = il[i]
                si = inst.sync_info
                if si is not None and len(si.on_wait) > 1:
                    waits = list(si.on_wait)
                    inst.sync_info = bass_rust.SyncInfo(
                        on_wait=[waits[-1]], on_update=list(si.on_update)
                    )
                    for w in waits[:-1]:
                        nop = mybir.InstNoOp(
                            name=f"WSPLIT-{uid}",
                            engine=inst.engine,
                            bass_nofuse=True,
                            sync_info=bass_rust.SyncInfo(
                                on_wait=[w], on_update=[]
                            ),
                        )
                        uid += 1
                        il.insert(i, nop)
                        i += 1
                i += 1


def bcast_ap(ap, parts, n):
    """Partition-broadcast view of a DRAM row AP: [[0,parts],[1,n]]."""
    return bass.AP(tensor=ap.tensor, offset=ap.offset, ap=[[0, parts], [1, n]])


def build_kernel():
    nc = bass.Bass(trn_type="TRN2")

    xq = nc.dram_tensor("xq", (D, S), MMD, kind="ExternalInput")   # query[b].T
    xk = nc.dram_tensor("xk", (D, S), MMD, kind="ExternalInput")
    xv = nc.dram_tensor("xv", (D, S), MMD, kind="ExternalInput")
    wq = nc.dram_tensor("wq", (D, DHG), MMD, kind="ExternalInput")  # Wq[hg].T
    wk = nc.dram_tensor("wk", (D, DHG), MMD, kind="ExternalInput")
    wv = nc.dram_tensor("wv", (D, DHG), MMD, kind="ExternalInput")
    wo = nc.dram_tensor("wo", (DHG, D), MMD, kind="ExternalInput")  # Wo[:,hg].T
    bq = nc.dram_tensor("bq", (DHG,), F32, kind="ExternalInput")
    bk = nc.dram_tensor("bk", (DHG,), F32, kind="ExternalInput")
    bv = nc.dram_tensor("bv", (DHG,), F32, kind="ExternalInput")
    out = nc.dram_tensor("out", (S, D), F32, kind="ExternalOutput")

    from contextlib import ExitStack

    with tile.TileContext(nc) as tc, ExitStack() as ctx:
        persist = ctx.enter_context(tc.tile_pool(name="persist", bufs=1))
        KT_sb = persist.tile([128, NP, S], MMD)       # K^T: pair p rows
        QT_sb = persist.tile([128, NP, S], MMD)       # Q^T
        V_sb = persist.tile([128, KT, HG, VW], MMD)   # V token-major + ones
        AON = persist.tile([128, NP, S], MMD)         # normalized AO^T
        bq_sb = persist.tile([128, NP], F32)
        bk_sb = persist.tile([128, NP], F32)
        bv_bc = persist.tile([128, DHG], F32)          # bv broadcast

        with nc.allow_non_contiguous_dma(reason="tiny bias loads"):
            nc.sync.dma_start(bq_sb[:], bq.rearrange("(t p) -> p t", p=128))
            nc.sync.dma_start(bk_sb[:], bk.rearrange("(t p) -> p t", p=128))
        nc.sync.dma_start(bv_bc[:], bcast_ap(bv[:], 128, DHG))
        if USE_BF16:
            nc.vector.memset(V_sb[:, :, :, DK], 1.0)           # ones columns
        else:
            nc.vector.memset(V_sb[:, :, :, DK].bitcast(F32), 1.0)

        # ---- phase A: projections ------------------------------------------
        phase_a = ExitStack()
        wpool = phase_a.enter_context(tc.tile_pool(name="wts", bufs=1))
        xpool = phase_a.enter_context(tc.tile_pool(name="xstream", bufs=3))
        pproj = phase_a.enter_context(
            tc.tile_pool(name="pproj", bufs=1, space="PSUM")
        )

        wk_sb = wpool.tile([128, CT, DHG], MMD)
        wq_sb = wpool.tile([128, CT, DHG], MMD)
        wv_sb = wpool.tile([128, CT, DHG], MMD)
        nc.sync.dma_start(wk_sb[:], wk.rearrange("(c p) n -> p c n", p=128))
        nc.sync.dma_start(wq_sb[:], wq.rearrange("(c p) n -> p c n", p=128))
        nc.sync.dma_start(wv_sb[:], wv.rearrange("(c p) n -> p c n", p=128))

        def kq_pass(xdram, w_sb, dst, b_sb, jts, khs):
            """dst[dout 128, q] += w[:,ct,jt].T @ x^T[ct, q] for jt in jts."""
            for kh in khs:
                ps = {
                    jt: pproj.tile([128, QB], F32, tag=f"proj{jt}",
                                   name=f"proj{jt}")
                    for jt in jts
                }
                for ct in range(CT):
                    xc = xpool.tile([128, QB], MMD, tag="xchunk", name="xc")
                    nc.sync.dma_start(
                        xc[:], xdram[ct * 128:(ct + 1) * 128,
                                     kh * QB:(kh + 1) * QB]
                    )
                    for jt in jts:
                        for qc in range(QB // 512):
                            nc.tensor.matmul(
                                ps[jt][:, qc * 512:(qc + 1) * 512],
                                w_sb[:, ct, jt * 128:(jt + 1) * 128],
                                xc[:, qc * 512:(qc + 1) * 512],
                                start=(ct == 0), stop=(ct == CT - 1),
                            )
                for jt in jts:
                    nc.vector.tensor_scalar_add(
                        dst[:, jt, kh * QB:(kh + 1) * QB],
                        ps[jt][:],
                        b_sb[:, jt:jt + 1],
                    )

        def v_pass():
            """V_sb[tok, h, 0:64] += x^T[ct, tok].T @ wv[:, ct, :], + bias."""
            for q4 in range(4):  # quarters of the token dim (4 tok-tiles)
                ps = {
                    i: pproj.tile([128, DHG], F32, tag=f"proj{i}",
                                  name=f"vproj{i}")
                    for i in range(4)
                }
                for ct in range(CT):
                    xc = xpool.tile([128, 512], MMD, tag="xchunk", name="xcv")
                    nc.sync.dma_start(
                        xc[:], xv[ct * 128:(ct + 1) * 128,
                                  q4 * 512:(q4 + 1) * 512]
                    )
                    for i in range(4):
                        nc.tensor.matmul(
                            ps[i][:],
                            xc[:, i * 128:(i + 1) * 128],
                            wv_sb[:, ct, :],
                            start=(ct == 0), stop=(ct == CT - 1),
                        )
                for i in range(4):
                    vtile = q4 * 4 + i
                    nc.vector.tensor_add(
                        V_sb[:, vtile, :, 0:DK],
                        ps[i][:].rearrange("p (h d) -> p h d", d=DK),
                        bv_bc[:].rearrange("p (h d) -> p h d", d=DK),
                    )

        # pair-0 K and q-block-0 Q first so attention starts early
        kq_pass(xk, wk_sb, KT_sb, bk_sb, jts=[0], khs=[0, 1])
        kq_pass(xq, wq_sb, QT_sb, bq_sb, jts=[0], khs=[0])
        v_pass()
        kq_pass(xk, wk_sb, KT_sb, bk_sb, jts=[1, 2, 3], khs=[0, 1])
        kq_pass(xq, wq_sb, QT_sb, bq_sb, jts=[1, 2, 3], khs=[0])
        kq_pass(xq, wq_sb, QT_sb, bq_sb, jts=[0, 1, 2, 3], khs=[1])

        phase_a.close()

        # ---- phases B+C: attention + out-projection ------------------------
        wopool = ctx.enter_context(tc.tile_pool(name="wopool", bufs=1))
        wo_sb = wopool.tile([128, NP, D], MMD)        # out-proj weights
        nc.sync.dma_start(wo_sb[:], wo.rearrange("(c p) n -> p c n", p=128))

        pmm = ctx.enter_context(tc.tile_pool(name="pmm", bufs=1, space="PSUM"))
        ptp = ctx.enter_context(tc.tile_pool(name="ptile", bufs=5))
        npool = ctx.enter_context(tc.tile_pool(name="norm", bufs=3))
        opool = ctx.enter_context(tc.tile_pool(name="ostage", bufs=2))
        dpool = ctx.enter_context(
            tc.tile_pool(name="dscratch", bufs=3, space="DRAM")
        )

        def outproj_tile(qb, tt):
            """Emit out-projection for token tile tt of q-block qb."""
            q0 = qb * QB
            ot = opool.tile([128, D], F32, tag="ot", name="ot")
            po = pmm.tile([128, QB], F32, tag=f"ao{tt % 2}", name="po")
            for oh in range(2):
                for ci in range(NP):
                    nc.tensor.matmul(
                        po[:, oh * 512:(oh + 1) * 512],
                        AON[:, ci, q0 + tt * 128:q0 + (tt + 1) * 128],
                        wo_sb[:, ci, oh * 512:(oh + 1) * 512],
                        start=(ci == 0), stop=(ci == NP - 1),
                    )
            nc.vector.tensor_copy(ot[:], po[:])
            nc.sync.dma_start(out[q0 + tt * 128:q0 + (tt + 1) * 128, :], ot[:])

        for qb in range(NQB):
            q0 = qb * QB
            for p in range(NP):
                ao = [
                    pmm.tile([VW, QB], F32, tag=f"ao{h2}", name=f"ao{h2}")
                    for h2 in range(2)
                ]
                for kt in range(KT):
                    for h2 in range(2):
                        hh = 2 * p + h2
                        lo, hi = h2 * DK, h2 * DK + DK
                        st = pmm.tile([128, QB], F32, tag="st", name="st",
                                      bufs=2)
                        for qc in range(QB // 512):
                            nc.tensor.matmul(
                                st[:, qc * 512:(qc + 1) * 512],
                                KT_sb[lo:hi, p, kt * 128:(kt + 1) * 128],
                                QT_sb[lo:hi, p,
                                      q0 + qc * 512:q0 + (qc + 1) * 512],
                                start=True, stop=True,
                            )
                        pt = ptp.tile([128, QB], MMD, tag="pt", name="pt")
                        nc.scalar.activation(
                            pt[:], st[:],
                            mybir.ActivationFunctionType.Exp,
                            scale=0.125,
                        )
                        for qc in range(QB // 512):
                            nc.tensor.matmul(
                                ao[h2][:, qc * 512:(qc + 1) * 512],
                                V_sb[:, kt, hh, :],
                                pt[:, qc * 512:(qc + 1) * 512],
                                start=(kt == 0), stop=(kt == KT - 1),
                            )
                for h2 in range(2):
                    # copy to SBUF promptly: frees the PSUM slot so the next
                    # pair's PV stream never stalls the PE
                    aos = npool.tile([VW, QB], F32, tag="aos", name="aos")
                    nc.vector.tensor_copy(aos[:], ao[h2][:])
                    # full-lane reciprocal of the denominator via a
                    # [1,1024] -> [128,8] DRAM-bounce reshape
                    dn = dpool.tile([1, QB], F32, tag="dn", name="dn")
                    nc.sync.dma_start(dn[:], aos[DK:VW, :])
                    rc = npool.tile([128, 8], F32, tag="rc", name="rc")
                    nc.sync.dma_start(
                        rc[:], dn[:].rearrange("x (p j) -> (x p) j", j=8)
                    )
                    nc.vector.reciprocal(rc[:], rc[:])
                    rcd = dpool.tile([1, QB], F32, tag="rcd", name="rcd")
                    nc.sync.dma_start(
                        rcd[:].rearrange("x (p j) -> (x p) j", j=8), rc[:]
                    )
                    rb = npool.tile([DK, QB], F32, tag="rb", name="rb")
                    nc.sync.dma_start(rb[:], bcast_ap(rcd[:], DK, QB))
                    nc.vector.tensor_mul(
                        AON[h2 * DK:(h2 + 1) * DK, p, q0:q0 + QB],
                        aos[0:DK, :],
                        rb[:],
                    )
                # interleave previous q-block's out-projection
                if qb > 0:
                    for tt in range(2 * p, 2 * p + 2):
                        outproj_tile(qb - 1, tt)
        for tt in range(QB // 128):
            outproj_tile(NQB - 1, tt)

    split_multi_waits(nc)
    return nc


def _prep_inputs(query, key, value, Wq, bq, Wk, bk, Wv, bv, Wo, bo):
    """Build the 8 per-core input maps."""
    import ml_dtypes

    mmd = ml_dtypes.bfloat16 if USE_BF16 else np.float32

    def cvt(a):
        return np.ascontiguousarray(a.astype(mmd))

    xt = {}
    for nm, x in (("xq", query), ("xk", key), ("xv", value)):
        xt[nm] = [cvt(x[b].T) for b in range(B)]
    in_maps = []
    for c in range(8):
        b, g = divmod(c, 2)
        rows = slice(g * DHG, (g + 1) * DHG)
        in_maps.append({
            "xq": xt["xq"][b], "xk": xt["xk"][b], "xv": xt["xv"][b],
            "wq": cvt(Wq[rows, :].T),
            "wk": cvt(Wk[rows, :].T),
            "wv": cvt(Wv[rows, :].T),
            "wo": cvt(Wo[:, rows].T),
            "bq": np.ascontiguousarray(bq[rows]),
            "bk": np.ascontiguousarray(bk[rows]),
            "bv": np.ascontiguousarray(bv[rows]),
        })
    return in_maps


_NC_CACHE = None


def run(inputs, trace=False):
    """Returns (full_output, BassKernelResults)."""
    global _NC_CACHE
    from concourse.bass_utils import run_bass_kernel_spmd

    inputs = {k: np.asarray(v, np.float32) for k, v in inputs.items()}
    in_maps = _prep_inputs(**inputs)
    if _NC_CACHE is None:
        _NC_CACHE = build_kernel()
    res = run_bass_kernel_spmd(
        _NC_CACHE, in_maps, core_ids=list(range(8)), trace=trace
    )
    bo = inputs["bo"]
    full = np.empty((B, S, D), np.float32)
    for b in range(B):
        full[b] = res.results[2 * b]["out"] + res.results[2 * b + 1]["out"] + bo
    return full, res


def kernel(**inputs):
    return run(inputs, trace=False)[0]


# revision 18
# speedup vs baseline: 1.6317x; 1.4644x over previous
"""Multi-head attention (B=4, S=2048, D=1024, H=16) on 8 Trainium2 cores.

Sharding: (batch, head-group) grid — core c handles batch c//2, heads
(c%2)*8..(c%2)*8+8. Zero duplicated FLOPs; host sums the two partial
out-projections per batch and adds bo.

Per-core kernel. Matmul operands in fp16 (1 cy/row like bf16, but 10-bit
mantissa; accumulation is always fp32 in PSUM), everything else fp32.

Layouts (all on-chip, no transposes anywhere):
  K^T/Q^T [512, 2048] feature-major; V [tok, head, 65] token-major with a
  ones column per head; scores computed k-major: S^T[k,q] = K.Q^T, so the
  exp output IS P^T, and the ones column makes the PV matmul accumulate the
  softmax denominator in PSUM row 64.

Scheduling strategy (the HAM clock-gate makes idle PE drop to 1.2 GHz and
an ACT-paced pipeline never re-warms): attention uses only 6 PSUM banks
(st [128,2x512] double-buffered + two [65,512] accumulators), leaving a
2-bank "pj" tag that projection tiles and out-projection tiles share.
Projection work is emitted interleaved between attention iterations so the
scheduler can fill every PE gap with useful matmuls and the PE stays dense
(= stays at 2.4 GHz) while ScalarE streams the exps.
"""

import numpy as np

import bass_rust
import concourse.bass as bass
import concourse.tile as tile
from concourse import mybir

F32 = mybir.dt.float32
MMD = mybir.dt.float16     # matmul operand dtype

B, S, D = 4, 2048, 1024
NH, DK = 16, 64            # total heads, head dim
HG = 8                     # heads per core (head group)
DHG = HG * DK              # 512 features per head group
NP = 4                     # pairs of heads per core
QS = 512                   # q-slice size
NQS = S // QS              # 4
KT = S // 128              # 16 k-tiles
CT = D // 128              # 8 contraction chunks for projections
VW = DK + 1                # 65: V columns per head incl. ones column


def split_multi_waits(nc):
    """This toolchain's walrus accepts only ONE sync-wait per instruction;
    Tile attaches several (one per producer proc). Hoist all but one wait
    onto single-wait NOPs inserted just before the instruction on the same
    engine (engines are in-order, so semantics are identical)."""
    uid = 0
    for f in nc.m.functions:
        for bb in f.blocks:
            il = bb.instructions
            i = 0
            while i < len(il):
                inst = il[i]
                si = inst.sync_info
                if si is not None and len(si.on_wait) > 1:
                    waits = list(si.on_wait)
                    inst.sync_info = bass_rust.SyncInfo(
                        on_wait=[waits[-1]], on_update=list(si.on_update)
                    )
                    for w in waits[:-1]:
                        nop = mybir.InstNoOp(
                            name=f"WSPLIT-{uid}",
                            engine=inst.engine,
                            bass_nofuse=True,
                            sync_info=bass_rust.SyncInfo(
                                on_wait=[w], on_update=[]
                            ),
                        )
                        uid += 1
                        il.insert(i, nop)
                        i += 1
                i += 1


def bcast_ap(ap, parts, n):
    """Partition-broadcast view of a DRAM row AP: [[0,parts],[1,n]]."""
    return bass.AP(tensor=ap.tensor, offset=ap.offset, ap=[[0, parts], [1, n]])


def build_kernel():
    nc = bass.Bass(trn_type="TRN2")

    xq = nc.dram_tensor("xq", (D, S), MMD, kind="ExternalInput")   # query[b].T
    xk = nc.dram_tensor("xk", (D, S), MMD, kind="ExternalInput")
    xv = nc.dram_tensor("xv", (D, S), MMD, kind="ExternalInput")
    wq = nc.dram_tensor("wq", (D, DHG), MMD, kind="ExternalInput")  # Wq[hg].T
    wk = nc.dram_tensor("wk", (D, DHG), MMD, kind="ExternalInput")
    wv = nc.dram_tensor("wv", (D, DHG), MMD, kind="ExternalInput")
    wo = nc.dram_tensor("wo", (DHG, D), MMD, kind="ExternalInput")  # Wo[:,hg].T
    bq = nc.dram_tensor("bq", (DHG,), F32, kind="ExternalInput")
    bk = nc.dram_tensor("bk", (DHG,), F32, kind="ExternalInput")
    bv = nc.dram_tensor("bv", (DHG,), F32, kind="ExternalInput")
    out = nc.dram_tensor("out", (S, D), F32, kind="ExternalOutput")

    from contextlib import ExitStack

    with tile.TileContext(nc) as tc, ExitStack() as ctx:
        persist = ctx.enter_context(tc.tile_pool(name="persist", bufs=1))
        KT_sb = persist.tile([128, NP, S], MMD)        # K^T: pair p rows
        QT_sb = persist.tile([128, NP, S], MMD)        # Q^T
        V_sb = persist.tile([128, KT, HG, VW], MMD)    # V token-major + ones
        AON = persist.tile([128, NP, S], MMD)          # normalized AO^T
        wk_sb = persist.tile([128, CT, DHG], MMD)
        wq_sb = persist.tile([128, CT, DHG], MMD)
        wv_sb = persist.tile([128, CT, DHG], MMD)
        wo_sb = persist.tile([128, NP, D], MMD)
        bq_sb = persist.tile([128, NP], F32)
        bk_sb = persist.tile([128, NP], F32)
        bv_bc = persist.tile([128, DHG], F32)

        nc.sync.dma_start(wk_sb[:], wk.rearrange("(c p) n -> p c n", p=128))
        nc.sync.dma_start(wq_sb[:], wq.rearrange("(c p) n -> p c n", p=128))
        nc.sync.dma_start(wv_sb[:], wv.rearrange("(c p) n -> p c n", p=128))
        nc.sync.dma_start(wo_sb[:], wo.rearrange("(c p) n -> p c n", p=128))
        with nc.allow_non_contiguous_dma(reason="tiny bias loads"):
            nc.sync.dma_start(bq_sb[:], bq.rearrange("(t p) -> p t", p=128))
            nc.sync.dma_start(bk_sb[:], bk.rearrange("(t p) -> p t", p=128))
        nc.sync.dma_start(bv_bc[:], bcast_ap(bv[:], 128, DHG))
        nc.vector.memset(V_sb[:, :, :, DK], 1.0)       # ones columns

        pmm = ctx.enter_context(tc.tile_pool(name="pmm", bufs=1, space="PSUM"))
        xpool = ctx.enter_context(tc.tile_pool(name="xw", bufs=10))
        ptp = ctx.enter_context(tc.tile_pool(name="ptile", bufs=5))
        npool = ctx.enter_context(tc.tile_pool(name="norm", bufs=3))
        opool = ctx.enter_context(tc.tile_pool(name="ostage", bufs=2))
        dpool = ctx.enter_context(
            tc.tile_pool(name="dscratch", bufs=3, space="DRAM")
        )

        def window(xdram, qs):
            """Load the 8 contraction chunks of one 512-token slice."""
            win = []
            for ct in range(CT):
                xc = xpool.tile([128, QS], MMD, tag="xw", name="xw")
                nc.sync.dma_start(
                    xc[:], xdram[ct * 128:(ct + 1) * 128,
                                 qs * QS:(qs + 1) * QS]
                )
                win.append(xc)
            return win

        def kq_tile(xdram, w_sb, dst, b_sb, jt, qs):
            """dst[:, jt, qs] = w[:, :, jt].T @ x^T[:, qs] + bias."""
            win = window(xdram, qs)
            ps = pmm.tile([128, QS], F32, tag="pj", name="pj", bufs=2)
            for ct in range(CT):
                nc.tensor.matmul(
                    ps[:],
                    w_sb[:, ct, jt * 128:(jt + 1) * 128],
                    win[ct][:],
                    start=(ct == 0), stop=(ct == CT - 1),
                )
            nc.vector.tensor_scalar_add(
                dst[:, jt, qs * QS:(qs + 1) * QS], ps[:], b_sb[:, jt:jt + 1]
            )

        def v_tiles(qs):
            """V_sb tok-tiles for one 512-token slice (4 tiles)."""
            win = window(xv, qs)
            for i in range(4):
                tt = qs * 4 + i
                ps = pmm.tile([128, DHG], F32, tag="pj", name="pjv", bufs=2)
                for ct in range(CT):
                    nc.tensor.matmul(
                        ps[:],
                        win[ct][:, i * 128:(i + 1) * 128],
                        wv_sb[:, ct, :],
                        start=(ct == 0), stop=(ct == CT - 1),
                    )
                nc.vector.tensor_add(
                    V_sb[:, tt, :, 0:DK],
                    ps[:].rearrange("p (h d) -> p h d", d=DK),
                    bv_bc[:].rearrange("p (h d) -> p h d", d=DK),
                )

        def attention(p, qsb):
            """One head-pair over one 512-wide q-slice."""
            q0 = qsb * QS
            ao = [
                pmm.tile([VW, QS], F32, tag=f"ao{h2}", name=f"ao{h2}")
                for h2 in range(2)
            ]
            for ktp in range(KT // 2):
                for h2 in range(2):
                    hh = 2 * p + h2
                    lo, hi = h2 * DK, h2 * DK + DK
                    st2 = pmm.tile([128, 2, QS], F32, tag="st", name="st",
                                   bufs=2)
                    for j in range(2):
                        kt = 2 * ktp + j
                        nc.tensor.matmul(
                            st2[:, j, :],
                            KT_sb[lo:hi, p, kt * 128:(kt + 1) * 128],
                            QT_sb[lo:hi, p, q0:q0 + QS],
                            start=True, stop=True,
                        )
                    pt2 = ptp.tile([128, 2, QS], MMD, tag="pt", name="pt")
                    nc.scalar.activation(
                        pt2[:], st2[:],
                        mybir.ActivationFunctionType.Exp,
                        scale=0.125,
                    )
                    for j in range(2):
                        kt = 2 * ktp + j
                        nc.tensor.matmul(
                            ao[h2][:],
                            V_sb[:, kt, hh, :],
                            pt2[:, j, :],
                            start=(kt == 0), stop=(kt == KT - 1),
                        )
            for h2 in range(2):
                # copy to SBUF promptly so PSUM frees fast
                aos = npool.tile([VW, QS], F32, tag="aos", name="aos")
                nc.vector.tensor_copy(aos[:], ao[h2][:])
                # full-lane reciprocal via [1,512] -> [128,4] DRAM reshape
                dn = dpool.tile([1, QS], F32, tag="dn", name="dn")
                nc.sync.dma_start(dn[:], aos[DK:VW, :])
                rc = npool.tile([128, 4], F32, tag="rc", name="rc")
                nc.sync.dma_start(
                    rc[:], dn[:].rearrange("x (p j) -> (x p) j", j=4)
                )
                nc.vector.reciprocal(rc[:], rc[:])
                rcd = dpool.tile([1, QS], F32, tag="rcd", name="rcd")
                nc.sync.dma_start(
                    rcd[:].rearrange("x (p j) -> (x p) j", j=4), rc[:]
                )
                rb = npool.tile([DK, QS], F32, tag="rb", name="rb")
                nc.sync.dma_start(rb[:], bcast_ap(rcd[:], DK, QS))
                nc.vector.tensor_mul(
                    AON[h2 * DK:(h2 + 1) * DK, p, q0:q0 + QS],
                    aos[0:DK, :],
                    rb[:],
                )

        def outproj_tile(qsb, tt):
            """Out-projection for token tile tt (128 rows) of q-slice qsb."""
            q0 = qsb * QS
            ot = opool.tile([128, D], F32, tag="ot", name="ot")
            for oh in range(2):
                po = pmm.tile([128, 512], F32, tag="pj", name="po", bufs=2)
                for ci in range(NP):
                    nc.tensor.matmul(
                        po[:],
                        AON[:, ci, q0 + tt * 128:q0 + (tt + 1) * 128],
                        wo_sb[:, ci, oh * 512:(oh + 1) * 512],
                        start=(ci == 0), stop=(ci == NP - 1),
                    )
                nc.vector.tensor_copy(ot[:, oh * 512:(oh + 1) * 512], po[:])
            nc.sync.dma_start(out[q0 + tt * 128:q0 + (tt + 1) * 128, :], ot[:])

        # ---- emission schedule ---------------------------------------------
        # prerequisites for attention(p=0, qsb=0): V complete, K pair0 all
        # k, Q pair0 slice0. Everything else dribbles in as PE filler.
        for qs in range(NQS):
            v_tiles(qs)
        for qs in range(NQS):
            kq_tile(xk, wk_sb, KT_sb, bk_sb, 0, qs)
        kq_tile(xq, wq_sb, QT_sb, bq_sb, 0, 0)

        for qsb in range(NQS):
            for p in range(NP):
                attention(p, qsb)
                if qsb == 0 and p < NP - 1:
                    for qs in range(NQS):
                        kq_tile(xk, wk_sb, KT_sb, bk_sb, p + 1, qs)
                    kq_tile(xq, wq_sb, QT_sb, bq_sb, p + 1, 0)
                if qsb < NQS - 1:
                    kq_tile(xq, wq_sb, QT_sb, bq_sb, p, qsb + 1)
                if qsb > 0:
                    outproj_tile(qsb - 1, p)
        for tt in range(NQS):
            outproj_tile(NQS - 1, tt)

    split_multi_waits(nc)
    return nc


def _prep_inputs(query, key, value, Wq, bq, Wk, bk, Wv, bv, Wo, bo):
    """Build the 8 per-core input maps."""
    def cvt(a):
        return np.ascontiguousarray(a.astype(np.float16))

    xt = {}
    for nm, x in (("xq", query), ("xk", key), ("xv", value)):
        xt[nm] = [cvt(x[b].T) for b in range(B)]
    in_maps = []
    for c in range(8):
        b, g = divmod(c, 2)
        rows = slice(g * DHG, (g + 1) * DHG)
        in_maps.append({
            "xq": xt["xq"][b], "xk": xt["xk"][b], "xv": xt["xv"][b],
            "wq": cvt(Wq[rows, :].T),
            "wk": cvt(Wk[rows, :].T),
            "wv": cvt(Wv[rows, :].T),
            "wo": cvt(Wo[:, rows].T),
            "bq": np.ascontiguousarray(bq[rows]),
            "bk": np.ascontiguousarray(bk[rows]),
            "bv": np.ascontiguousarray(bv[rows]),
        })
    return in_maps


_NC_CACHE = None


def run(inputs, trace=False):
    """Returns (full_output, BassKernelResults)."""
    global _NC_CACHE
    from concourse.bass_utils import run_bass_kernel_spmd

    inputs = {k: np.asarray(v, np.float32) for k, v in inputs.items()}
    in_maps = _prep_inputs(**inputs)
    if _NC_CACHE is None:
        _NC_CACHE = build_kernel()
    res = run_bass_kernel_spmd(
        _NC_CACHE, in_maps, core_ids=list(range(8)), trace=trace
    )
    bo = inputs["bo"]
    full = np.empty((B, S, D), np.float32)
    for b in range(B):
        full[b] = res.results[2 * b]["out"] + res.results[2 * b + 1]["out"] + bo
    return full, res


def kernel(**inputs):
    return run(inputs, trace=False)[0]
